# revision 19
# baseline (speedup 1.0000x reference)
"""Trainium2 Bass kernel for multi-head attention (B=4, N=2048, C=256, H=16).

Sharding: 8 cores, each core handles one batch b = core//2 and 8 heads
(half of 16) g = core%2.  Each core computes its 8 heads' attention plus a
partial output projection (its heads' rows of w_proj); the host sums the
two partials per batch and adds b_proj.  x is transposed on the host so
the [channels-on-partitions] layout DMAs straight in (no on-chip
transposes for x).

Per-core on-chip algorithm (all layouts "transposed", channels on
partitions):
  qT/kT (spread layout: head j of a 4-head group occupies partitions
        32j..32j+16) = W^T @ xT, bias fused into the single PSUM->SBUF
        tensor_scalar_add                                     [128, N]
  vT   (compact: head lh at partitions 16lh)                  [128, N]
  v_aug[keys, lh, 0:16] = v, v_aug[keys, lh, 16] = 1          (ones col
        makes the attn@v matmul also produce softmax row-sums)
  S^T  = k_h @ q_h^T   (row-group-packed matmuls, K=16)       [keys, q]
  P^T  = exp(S^T): the exp units ([128,1024] tiles) are split THREE ways
        across ScalarE (table exp, ~153 G/s), VectorE and GPSIMD (both
        Schraudolph: one fused tensor_scalar bits =
        round(128*log2e*s + (128*127-7.4)) as int16, whose bit pattern
        IS bf16(exp(s)) -- rel err rms ~1.8%; consumed via a free
        bitcast).  Assignment is a greedy min-completion-time schedule
        over modeled per-op engine costs.
  outT_aug = v_aug^T @ P^T  accumulated over key tiles; ALL FOUR head
        chains share ONE psum bank: a full-width [128,512] zero matmul
        with start=True zero-fills the bank (has_written set for every
        element), every chain matmul then runs start=False and
        accumulates onto the zeros.  Row 16 of each 32-row col-group =
        sum_j P^T[j, q] (softmax denominator); unwritten rows stay 0.
  bc   = Sel^T @ outT  broadcasts each group's sum row over the group
  outT_norm = outT * reciprocal_approx_fast(bc)
  partial = outT_norm^T @ Wp_spread   (zero rows kill sum/garbage rows)

SOFTWARE PIPELINING (the key to performance): the PE is an in-order
engine.  The naive emission order sc(kt) -> exp(kt) -> av(kt) ->
sc(kt+1) serializes every kt iteration behind the ~1.1us exp latency
(~2.2us/kt fully serial).  Instead the attn@v matmul for kt is emitted
AV_LAG iterations behind the scores matmul, so the PE streams ahead
filling the 3-tile PSUM score ring while ScalarE/VectorE/GPSIMD drain
exp tiles in parallel; steady state is exp-throughput-bound, not
latency-bound.  The normalize/projection tail of each (nn,g2) group is
likewise deferred into the next group's kt loop (DEFER_TAIL) so its
PE ops (bc, proj) don't stall on V/S queue latency.

PSUM budget (8 banks): 3 x [128,1024] score tiles (6 banks, also reused
as stage-A qkv/v-transpose scratch) + 1 shared attnv accumulator bank +
1 bank for the bc/pp normalize/projection tail.

Matmul dtypes: bf16 throughout the matmul path (FWL fast-weight-load
only engages for non-fp32 weights).
"""

import os

import numpy as np

import concourse.bass as bass
import concourse.mybir as mybir
import concourse.tile as tile
from concourse import bacc

F32 = mybir.dt.float32
F32R = mybir.dt.float32r
BF16 = mybir.dt.bfloat16
I16 = mybir.dt.int16
EXPF = mybir.ActivationFunctionType.Exp
COPYF = mybir.ActivationFunctionType.Copy

P = 128
B, N_FULL, C, H, D = 4, 2048, 256, 16, 16
CC = C // P  # 2 channel tiles
NCORES = 8

MM_DT = "bf16"    # qkv projection matmuls + x/weights (FWL weight path)
AV_DT = "bf16"    # attnv matmuls (col-group tile_position needs bf16 weights)
PROJ_DT = "bf16"  # sum-broadcast + output projection matmuls (FWL weight path)

_DT = {"f32r": F32R, "f32": F32, "bf16": BF16}

# Schraudolph int16/bf16 exp constants: bits = round(SC_A*s + SC_B);
# bitcast(bits) ~= exp(s), rel err rms ~1.8%, max ~4.2% (HW-validated).
_LOG2E = 1.4426950408889634
SC_A = 128.0 * _LOG2E
SC_B = 128.0 * 127.0 - 7.4

USE_DMA_T = True  # x transpose via DMA xbar (host passes x pre-transposed)

# EXP_MODE: "real" (normal) | "none" (timing ablation: attnv consumes a
# constant SBUF tile; output is numerically WRONG, only for attribution)
EXP_MODE = os.environ.get("EXP_MODE", "real")
TAIL_MODE = os.environ.get("TAIL_MODE", "real")  # "none": timing ablation
# Score-tile PSUM dtype: bf16 halves the tile to 1 bank (ring of 6 = 3kt
# of lookahead) and makes every exp operand 2-byte, enabling the DVE
# 2x_1p fast mode even from PSUM.  Scores are single-shot matmuls (no
# cross-matmul accumulation), so bf16 PSUM only costs output rounding.
SC_DT = os.environ.get("SC_DT", "f32")  # bf16 PSUM needs TRN3
# Score-tile granularity: "half" = per-head [128,512] tiles (1 PSUM bank
# each, ring of 6, finer head-of-line); "full" = [128,1024] 2-head tiles.
SC_GRAN = os.environ.get("SC_GRAN", "half")
# Exp-tile engine assignment: "greedy" (min completion time) or "alt"
# (strict S/V alternation per kt: avoids ring head-of-line convoys).
EXP_ASSIGN = os.environ.get("EXP_ASSIGN", "alt")

# ---- pipeline / scheduling knobs ----
AV_LAG = 2        # attnv matmuls trail the scores matmuls by this many kt
DEFER_TAIL = True       # emit each group's normalize/proj tail inside the
                        # next group's kt loop (hides PE tail stalls)
FLUSH_KT = 4      # kt index in the next group at which the tail is emitted

# Modeled per-op engine-busy costs (ns) used by the greedy scheduler.
# S=ScalarE(Act) V=VectorE(DVE) G=GPSIMD(Pool).  GPSIMD cannot access
# PSUM, so every op with a PSUM operand is restricted to S/V; GPSIMD
# gets the SBUF-only normalize multiply.
EXP_COST = {"S": 1040, "V": 1190}
BIAS_COST = {"S": 612}  # keep V pure-exp: V is the critical engine
VAUG_COST = {"S": 292}
ATCP_COST = {"S": 612}
OB_COST = {"S": 398}
REC_COST = 660
MUL_COST = {"V": 660, "G": 1111}

_NC_CACHE: dict = {}
LAST_RESULT = None  # BassKernelResults of the most recent run (for test.py)
TIMING_REPS = 1  # >1 repeats the compute on-device (timing); output unchanged


class _Sched:
    """Greedy min-completion-time assignment of elementwise ops to the
    three non-PE compute engines, using the modeled costs above."""

    def __init__(self):
        self.busy = {"S": 0.0, "V": 0.0, "G": 0.0}

    def pick(self, costs):
        e = min(costs, key=lambda k: self.busy[k] + costs[k])
        self.busy[e] += costs[e]
        return e

    def charge(self, e, cost):
        self.busy[e] += cost


def build(n_tokens=N_FULL, reps=1):
    N = n_tokens
    KT = N // P   # key tiles
    QC = 512      # q-chunk (psum bank = 512 fp32)
    NQ = N // QC
    TT = N // P   # token tiles

    MD = _DT[MM_DT]
    AD = _DT[AV_DT]
    PD = _DT[PROJ_DT]

    nc = bacc.Bacc()
    if USE_DMA_T:
        # host passes x already transposed: [C, N]
        x_d = nc.dram_tensor("x", [C, N], MD, kind="ExternalInput")
    else:
        x_d = nc.dram_tensor("x", [N, C], F32, kind="ExternalInput")
    wq_d = nc.dram_tensor("wq", [2, C, P], MD, kind="ExternalInput")
    wk_d = nc.dram_tensor("wk", [2, C, P], MD, kind="ExternalInput")
    wv_d = nc.dram_tensor("wv", [C, P], MD, kind="ExternalInput")
    bq_d = nc.dram_tensor("bq", [2, P], F32, kind="ExternalInput")
    bk_d = nc.dram_tensor("bk", [2, P], F32, kind="ExternalInput")
    bv_d = nc.dram_tensor("bv", [P], F32, kind="ExternalInput")
    wp_d = nc.dram_tensor("wp", [2, P, C], PD, kind="ExternalInput")
    sel_d = nc.dram_tensor("sel", [P, P], PD, kind="ExternalInput")
    idn_d = nc.dram_tensor("idn", [P, P], F32, kind="ExternalInput")
    out_d = nc.dram_tensor("out", [N, C], F32, kind="ExternalOutput")

    with tile.TileContext(nc) as tc:
        with (
            tc.tile_pool(name="const", bufs=1) as const,
            tc.tile_pool(name="work", bufs=12) as work,
            tc.tile_pool(name="ptp", bufs=(20 if SC_GRAN == "half" else 10)) as ptp,
            # Ring of score tiles plus stage-A psum scratch (qkv,
            # v-transpose).  "half" granularity: 1-bank tiles, ring of 6.
            tc.tile_pool(name="ps_s", bufs=(6 if SC_GRAN == "half" else 3),
                         space="PSUM") as ps_s,
            # All 4 attnv accumulation chains share ONE bank.
            tc.tile_pool(name="ps_at", bufs=1, space="PSUM") as ps_at,
            # bc/pp normalize+projection scratch: keeps the tail off the
            # hot sc ring.
            tc.tile_pool(name="ps_sm", bufs=1, space="PSUM") as ps_sm,
        ):
            # ---------------- loads ----------------
            def staged_load(name, shape, dt, src_ap):
                sb = const.tile(shape, dt, name=f"{name}_sb")
                nc.sync.dma_start(sb[:], src_ap)
                return sb

            # Small constants first: the first qkv matmuls wait on the
            # weights — queuing them behind the 2 MB x transfer costs
            # ~10 us of PE idle at startup.
            idn_sb = staged_load("idn", [P, P], F32, idn_d[:])
            wq_sb = staged_load(
                "wq", [P, 2, CC, P], MD,
                wq_d[:].rearrange("g (cc p) f -> p g cc f", p=P),
            )
            wk_sb = staged_load(
                "wk", [P, 2, CC, P], MD,
                wk_d[:].rearrange("g (cc p) f -> p g cc f", p=P),
            )
            wv_sb = staged_load(
                "wv", [P, CC, P], MD, wv_d[:].rearrange("(cc p) f -> p cc f", p=P)
            )
            if USE_DMA_T:
                # x arrives host-transposed [C, N]: DMA straight into the
                # [channels-on-partitions] layout, no on-chip transposes.
                # Chunk 0 queued before the remaining small constants so
                # the first projections unblock as early as possible.
                xt_full = const.tile([P, CC, N], MD, name="xt_full")
                x_r = x_d[:].rearrange("(cc p) t -> p cc t", p=P)
                for cc in range(CC):
                    nc.sync.dma_start(
                        xt_full[:, cc, 0:QC], x_r[:, cc, 0:QC]
                    )
            bq_sb = staged_load("bq", [P, 2], F32, bq_d[:].rearrange("g p -> p g"))
            bk_sb = staged_load("bk", [P, 2], F32, bk_d[:].rearrange("g p -> p g"))
            bv_sb = staged_load(
                "bv", [P, 1], F32, bv_d[:].rearrange("(p o) -> p o", o=1)
            )
            wp_sb = staged_load("wp", [P, 2, C], PD, wp_d[:].rearrange("g p c -> p g c"))
            sel_sb = staged_load("sel", [P, P], PD, sel_d[:])

            if USE_DMA_T:
                for cc in range(CC):
                    for qq in range(1, NQ):
                        nc.sync.dma_start(
                            xt_full[:, cc, qq * QC : (qq + 1) * QC],
                            x_r[:, cc, qq * QC : (qq + 1) * QC],
                        )
                x_sb = None
            else:
                x_sb = const.tile([P, TT, C], F32)
                x_r = x_d[:].rearrange("(t p) c -> p t c", p=P)
                for tt in range(TT):
                    nc.sync.dma_start(x_sb[:, tt, :], x_r[:, tt, :])
                xt_full = None

            from contextlib import nullcontext

            loop_ctx = tc.For_i(0, reps, 1) if reps > 1 else nullcontext()
            with loop_ctx:
                _build_body(
                    nc, tc, const, work, ptp, ps_s, ps_at, ps_sm,
                    N, KT, QC, NQ, TT, MD, AD, PD,
                    x_sb, xt_full, wq_sb, wk_sb, wv_sb, wp_sb, sel_sb, idn_sb,
                    bq_sb, bk_sb, bv_sb, out_d,
                )
    nc.finalize()
    return nc


def _build_body(
    nc, tc, const, work, ptp, ps_s, ps_at, ps_sm,
    N, KT, QC, NQ, TT, MD, AD, PD,
    x_sb, xt_full, wq_sb, wk_sb, wv_sb, wp_sb, sel_sb, idn_sb,
    bq_sb, bk_sb, bv_sb, out_d,
):
    ps_m = ps_s
    sched = _Sched()
    ones_sb = const.tile([P, 1], F32)
    nc.vector.memset(ones_sb[:], 1.0)
    # bf16 zero row + ones row for the full-width at-bank zero-fill matmul
    # (bf16 keeps the 512-col clear at 1 cycle/row on the PE).
    zrow_sb = const.tile([1, P], BF16)
    nc.vector.memset(zrow_sb[:], 0.0)
    onesrow_sb = const.tile([1, QC], BF16)
    nc.vector.memset(onesrow_sb[:], 1.0)
    KC = QC // P  # key tiles per chunk
    # q/k stored bf16: scores matmuls then use the FWL bf16 weight path.
    qt_t = [const.tile([P, 2, QC], AD, name=f"qt{c}") for c in range(NQ)]
    kt_t = [const.tile([P, 2, QC], AD, name=f"kt{c}") for c in range(NQ)]
    vt_t = [const.tile([P, QC], F32, name=f"vt{c}") for c in range(NQ)]
    vaug_t = [
        const.tile([P, KC, 8, 17], AD, name=f"vaug{c}") for c in range(NQ)
    ]
    if xt_full is None:
        xt_t = [const.tile([P, CC, QC], MD, name=f"xt{c}") for c in range(NQ)]
    else:
        xt_t = None

    # ot_raw ping-pong buffers (written by a full [128,512] copy of the
    # at bank; rows the chains never touched are exact zeros from the
    # full-width clear matmul).
    ot_raw_pp = [const.tile([P, QC], PD, name=f"otraw{i}") for i in range(2)]

    def xt_ap(c):
        if xt_full is not None:
            return xt_full[:, :, c * QC : (c + 1) * QC]
        return xt_t[c][:]

    def emit_bias_copy(dslice, ps, b_ap):
        e = sched.pick(BIAS_COST)
        if e == "S":
            # Identity (not Copy) accepts an AP bias; both live in the same
            # activation-table set as Exp, so no table-swap cost.
            nc.scalar.activation(
                dslice, ps[:], mybir.ActivationFunctionType.Identity, bias=b_ap
            )
        elif e == "V":
            nc.vector.tensor_scalar_add(dslice, ps[:], b_ap)
        else:
            nc.gpsimd.tensor_scalar_add(dslice, ps[:], b_ap)

    def emit_copy(dst, src, costs):
        e = sched.pick(costs)
        if e == "S":
            nc.scalar.activation(dst, src, COPYF)
        elif e == "V":
            nc.vector.tensor_copy(dst, src)
        else:
            nc.gpsimd.tensor_copy(dst, src)

    for c in range(NQ):
        if xt_full is None:
            # xT for this chunk via PE transpose
            for ti in range(QC // P):
                tt = c * (QC // P) + ti
                for cc in range(CC):
                    tp = ps_m.tile([P, P], F32, tag="scores", name="tp")
                    nc.tensor.transpose(
                        tp[:], x_sb[:, tt, cc * P : (cc + 1) * P], idn_sb[:]
                    )
                    nc.vector.tensor_copy(
                        xt_t[c][:, cc, ti * P : (ti + 1) * P], tp[:]
                    )
        xc = xt_ap(c)
        # k, v (needed for all q-chunks) then q projections
        projs = [
            (wk_sb[:, 0], bk_sb[:, 0:1], kt_t[c][:, 0]),
            (wk_sb[:, 1], bk_sb[:, 1:2], kt_t[c][:, 1]),
            (wv_sb[:], bv_sb[:, 0:1], vt_t[c][:]),
            (wq_sb[:, 0], bq_sb[:, 0:1], qt_t[c][:, 0]),
            (wq_sb[:, 1], bq_sb[:, 1:2], qt_t[c][:, 1]),
        ]
        for w_ap, b_ap, dslice in projs:
            ps = ps_m.tile([P, QC], F32, tag="scores", name="ps")
            for cc in range(CC):
                nc.tensor.matmul(
                    ps[:],
                    w_ap[:, cc, :],
                    xc[:, cc, :],
                    start=(cc == 0),
                    stop=(cc == CC - 1),
                )
            # single fused PSUM->SBUF copy + per-partition bias add
            emit_bias_copy(dslice, ps, b_ap)
        # v_aug for this chunk (v natural layout + ones column)
        nc.vector.tensor_copy(
            vaug_t[c][:, :, :, 16],
            ones_sb[:, 0:1, None].to_broadcast((P, KC, 8)),
        )
        for ki in range(KC):
            tp = ps_m.tile([P, P], F32, tag="scores", name="tp")
            nc.tensor.transpose(
                tp[:], vt_t[c][:, ki * P : (ki + 1) * P], idn_sb[:]
            )
            emit_copy(
                vaug_t[c][:, ki, :, 0:16],
                tp[:].rearrange("p (h d) -> p h d", d=16),
                VAUG_COST,
            )

    # ---------------- attention ----------------
    # Per (nn, g2) group: software-pipelined kt loop.  PE emission order:
    #   clear, sc(0), sc(1), sc(2), av(0), sc(3), av(1), ...  (AV_LAG=2)
    # so the in-order PE never blocks behind the exp of the freshest tile.
    # The normalize/projection tail of the previous group is emitted at
    # kt==FLUSH_KT of the current group (DEFER_TAIL).
    pending_tail = []  # list of small closures, flushed one per kt slot

    def flush_tail():
        while pending_tail:
            pending_tail.pop(0)()

    def flush_one():
        if pending_tail:
            pending_tail.pop(0)()

    SCD = BF16 if SC_DT == "bf16" else F32

    def emit_sc(nn, g2, kt):
        if SC_GRAN == "half":
            scs = []
            for lj in range(4):
                rg = 32 * lj
                sc = ps_s.tile([P, QC], SCD, tag="scores", name="sc")
                nc.tensor.matmul(
                    sc[:],
                    kt_t[kt // KC][
                        rg : rg + D, g2,
                        (kt % KC) * P : (kt % KC + 1) * P,
                    ],
                    qt_t[nn][rg : rg + D, g2, :],
                    start=True,
                    stop=True,
                    tile_position=(rg, 0),
                )
                scs.append(sc)
            return scs
        scs = []
        for pr in range(2):
            sc = ps_s.tile([P, 2 * QC], SCD, tag="scores", name="sc")
            for j2 in range(2):
                lj = 2 * pr + j2
                rg = 32 * lj
                nc.tensor.matmul(
                    sc[:, j2 * QC : (j2 + 1) * QC],
                    kt_t[kt // KC][
                        rg : rg + D, g2,
                        (kt % KC) * P : (kt % KC + 1) * P,
                    ],
                    qt_t[nn][rg : rg + D, g2, :],
                    start=True,
                    stop=True,
                    tile_position=(rg, 0),
                )
            scs.append(sc)
        return scs

    dummy_pt = None
    if EXP_MODE == "none":
        dummy_pt = const.tile([P, 2 * QC], AD, name="dummy_pt")
        nc.vector.memset(dummy_pt[:], 0.001)
        if SC_GRAN == "half":
            dummy_pt = dummy_pt[:, 0:QC]

    def emit_exp(sc, pr=0):
        if EXP_MODE == "none":
            return dummy_pt[:]
        wid = QC if SC_GRAN == "half" else 2 * QC
        if EXP_ASSIGN == "alt":
            e = "S" if pr % 2 == 0 else "V"
            sched.charge(e, EXP_COST[e])
        else:
            e = sched.pick(EXP_COST)
        if e == "S":
            pt = ptp.tile([P, wid], AD, tag="pt", name="pt")
            nc.scalar.activation(pt[:], sc[:], EXPF)
            return pt[:]
        pt = ptp.tile([P, wid], I16, tag="pt", name="pt")
        eng = nc.vector if e == "V" else nc.gpsimd
        eng.tensor_scalar(
            pt[:], sc[:], SC_A, SC_B,
            mybir.AluOpType.mult, mybir.AluOpType.add,
        )
        return pt[:].bitcast(BF16)

    def emit_av(at, kt, pts):
        for lj in range(4):
            if SC_GRAN == "half":
                mv = pts[lj]
            else:
                pr, j2 = lj // 2, lj % 2
                mv = pts[pr][:, j2 * QC : (j2 + 1) * QC]
            nc.tensor.matmul(
                at[32 * lj : 32 * lj + 17, :],
                vaug_t[kt // KC][:, kt % KC, 4 * g2_of_av[0] + lj, :],
                mv,
                start=False,
                stop=(kt == KT - 1),
                tile_position=(0, 32 * lj),
            )

    g2_of_av = [0]  # mutable holder so emit_av sees the current group's g2

    for nn in range(NQ):
        ot_n = work.tile([P, 2, QC], PD, tag="otn")
        for g2 in range(2):
            g2_of_av[0] = g2
            # Zero-fill the whole at bank with a full-width start=True
            # matmul: every element gets has_written=1 and value 0, so
            # all four 17-row chains accumulate with start=False and the
            # untouched rows read back as exact zeros.
            at = ps_at.tile([P, QC], F32, tag="at", name="at")
            nc.tensor.matmul(
                at[:], zrow_sb[:], onesrow_sb[:], start=True, stop=True,
            )
            pend = []  # (kt, pts) awaiting their attnv matmuls
            for kt in range(KT):
                scs = emit_sc(nn, g2, kt)
                pts = [emit_exp(sc_i, i) for i, sc_i in enumerate(scs)]
                pend.append((kt, pts))
                if kt >= 2 and kt % 2 == 0:
                    flush_one()
                if len(pend) > AV_LAG:
                    k0, p0 = pend.pop(0)
                    emit_av(at, k0, p0)
            for k0, p0 in pend:
                emit_av(at, k0, p0)

            # ---- tail: single full-bank PSUM->SBUF copy now (frees the
            # at bank for the next group), normalize+projection deferred.
            if TAIL_MODE == "none":
                continue
            ot_raw = ot_raw_pp[(2 * nn + g2) % 2]
            emit_copy(ot_raw[:], at[:], ATCP_COST)

            def mk_norm(nn=nn, g2=g2, ot_raw=ot_raw, ot_n=ot_n):
                def norm():
                    bc = ps_sm.tile([P, QC], F32, tag="small", name="bc")
                    nc.tensor.matmul(
                        bc[:], sel_sb[:], ot_raw[:], start=True, stop=True
                    )
                    rec = work.tile([P, QC], F32, tag="rec")
                    nc.vector.reciprocal_approx_fast(rec[:], bc[:])
                    sched.charge("V", REC_COST)
                    e = sched.pick(MUL_COST)
                    eng = nc.vector if e == "V" else nc.gpsimd
                    eng.tensor_mul(ot_n[:, g2, :], ot_raw[:], rec[:])
                return norm

            def mk_proj(nn=nn, ot_n=ot_n, ss=0):
                def proj():
                    pp = ps_sm.tile([P, C], F32, tag="small", name="pp")
                    for gg in range(2):
                        nc.tensor.matmul(
                            pp[:],
                            ot_n[:, gg, ss * P : (ss + 1) * P],
                            wp_sb[:, gg, :],
                            start=(gg == 0),
                            stop=(gg == 1),
                        )
                    ob = work.tile([P, C], F32, tag="ob")
                    emit_copy(ob[:], pp[:], OB_COST)
                    tt_idx = nn * (QC // P) + ss
                    nc.sync.dma_start(
                        out_d[:].rearrange("(t p) c -> p t c", p=P)[
                            :, tt_idx, :
                        ],
                        ob[:],
                    )
                return proj

            pending_tail.append(mk_norm())
            if g2 == 1:
                for ss in range(QC // P):
                    pending_tail.append(mk_proj(ss=ss))
            if not DEFER_TAIL:
                flush_tail()
    flush_tail()


def _get_nc(n_tokens=N_FULL, reps=1):
    key = (n_tokens, MM_DT, AV_DT, PROJ_DT, USE_DMA_T, reps,
           AV_LAG, DEFER_TAIL, FLUSH_KT, EXP_MODE, TAIL_MODE, SC_DT, EXP_ASSIGN,
           SC_GRAN,
           tuple(sorted(EXP_COST.items())))
    if key not in _NC_CACHE:
        _NC_CACHE[key] = build(n_tokens, reps=reps)
    return _NC_CACHE[key]


def make_core_inputs(core, x, w_qkv, b_qkv, w_proj, n_tokens=N_FULL):
    """Host-side sharding: slice/spread weights for one core."""
    b, g = core // 2, core % 2
    wq_s = np.zeros((2, C, P), np.float32)
    wk_s = np.zeros((2, C, P), np.float32)
    bq_s = np.zeros((2, P), np.float32)
    bk_s = np.zeros((2, P), np.float32)
    wv_s = np.zeros((C, P), np.float32)
    bv_s = np.zeros((P,), np.float32)
    wp_s = np.zeros((2, P, C), np.float32)
    for g2 in range(2):
        for j in range(4):
            h = 8 * g + 4 * g2 + j
            sp = slice(32 * j, 32 * j + D)
            wq_s[g2, :, sp] = w_qkv[:, 0 * C + h * D : 0 * C + (h + 1) * D]
            wk_s[g2, :, sp] = w_qkv[:, 1 * C + h * D : 1 * C + (h + 1) * D]
            bq_s[g2, sp] = b_qkv[0 * C + h * D : 0 * C + (h + 1) * D]
            bk_s[g2, sp] = b_qkv[1 * C + h * D : 1 * C + (h + 1) * D]
            wp_s[g2, sp, :] = w_proj[h * D : (h + 1) * D, :]
    for lh in range(8):
        h = 8 * g + lh
        wv_s[:, 16 * lh : 16 * lh + 16] = w_qkv[:, 2 * C + h * D : 2 * C + (h + 1) * D]
        bv_s[16 * lh : 16 * lh + 16] = b_qkv[2 * C + h * D : 2 * C + (h + 1) * D]
    sel = np.zeros((P, P), np.float32)
    for j in range(4):
        sel[32 * j + 16, 32 * j : 32 * j + 32] = 1.0
    idn = np.eye(P, dtype=np.float32)

    def cast(a, stage_dt):
        if stage_dt == "bf16":
            import ml_dtypes
            return a.astype(ml_dtypes.bfloat16)
        return a.astype(np.float32)

    if USE_DMA_T:
        x_core = cast(np.ascontiguousarray(x[b, :n_tokens].T), MM_DT)
    else:
        x_core = np.ascontiguousarray(x[b, :n_tokens], dtype=np.float32)
    return {
        "x": x_core,
        "wq": cast(wq_s, MM_DT), "wk": cast(wk_s, MM_DT), "wv": cast(wv_s, MM_DT),
        "bq": bq_s, "bk": bk_s, "bv": bv_s,
        "wp": cast(wp_s, PROJ_DT), "sel": cast(sel, PROJ_DT), "idn": idn,
    }


def kernel(x, w_qkv, b_qkv, w_proj, b_proj):
    global LAST_RESULT
    from concourse.bass_utils import run_bass_kernel_spmd

    x = np.asarray(x, dtype=np.float32)
    w_qkv = np.asarray(w_qkv, dtype=np.float32)
    b_qkv = np.asarray(b_qkv, dtype=np.float32)
    w_proj = np.asarray(w_proj, dtype=np.float32)
    b_proj = np.asarray(b_proj, dtype=np.float32)

    nc = _get_nc(reps=TIMING_REPS)
    in_maps = [
        make_core_inputs(core, x, w_qkv, b_qkv, w_proj) for core in range(NCORES)
    ]
    res = run_bass_kernel_spmd(nc, in_maps, list(range(NCORES)))
    LAST_RESULT = res
    out = np.zeros((B, N_FULL, C), np.float32)
    for core in range(NCORES):
        out[core // 2] += res.results[core]["out"]
    out += b_proj[None, None, :]
    return out


# revision 22
# speedup vs baseline: 1.0984x; 1.0984x over previous
"""Trainium2 Bass kernel for multi-head attention (B=4, N=2048, C=256, H=16).

Sharding: 8 cores, each core handles one batch b = core//2 and 8 heads
(half of 16) g = core%2.  Each core computes its 8 heads' attention plus a
partial output projection (its heads' rows of w_proj); the host sums the
two partials per batch and adds b_proj.  x is transposed on the host so
the [channels-on-partitions] layout DMAs straight in (no on-chip
transposes for x).

Per-core on-chip algorithm (all layouts "transposed", channels on
partitions):
  qT/kT (spread layout: head j of a 4-head group occupies partitions
        32j..32j+16) = W^T @ xT, bias fused into the single PSUM->SBUF
        tensor_scalar_add                                     [128, N]
  vT   (compact: head lh at partitions 16lh)                  [128, N]
  v_aug[keys, lh, 0:16] = v, v_aug[keys, lh, 16] = 1          (ones col
        makes the attn@v matmul also produce softmax row-sums)
  S^T  = k_h @ q_h^T   (row-group-packed matmuls, K=16)       [keys, q]
  P^T  = exp(S^T): the exp units ([128,1024] tiles) are split THREE ways
        across ScalarE (table exp, ~153 G/s), VectorE and GPSIMD (both
        Schraudolph: one fused tensor_scalar bits =
        round(128*log2e*s + (128*127-7.4)) as int16, whose bit pattern
        IS bf16(exp(s)) -- rel err rms ~1.8%; consumed via a free
        bitcast).  Assignment is a greedy min-completion-time schedule
        over modeled per-op engine costs.
  outT_aug = v_aug^T @ P^T  accumulated over key tiles; ALL FOUR head
        chains share ONE psum bank: a full-width [128,512] zero matmul
        with start=True zero-fills the bank (has_written set for every
        element), every chain matmul then runs start=False and
        accumulates onto the zeros.  Row 16 of each 32-row col-group =
        sum_j P^T[j, q] (softmax denominator); unwritten rows stay 0.
  bc   = Sel^T @ outT  broadcasts each group's sum row over the group
  outT_norm = outT * reciprocal_approx_fast(bc)
  partial = outT_norm^T @ Wp_spread   (zero rows kill sum/garbage rows)

SOFTWARE PIPELINING (the key to performance): the PE is an in-order
engine.  The naive emission order sc(kt) -> exp(kt) -> av(kt) ->
sc(kt+1) serializes every kt iteration behind the ~1.1us exp latency
(~2.2us/kt fully serial).  Instead the attn@v matmul for kt is emitted
AV_LAG iterations behind the scores matmul, so the PE streams ahead
filling the 3-tile PSUM score ring while ScalarE/VectorE/GPSIMD drain
exp tiles in parallel; steady state is exp-throughput-bound, not
latency-bound.  The normalize/projection tail of each (nn,g2) group is
likewise deferred into the next group's kt loop (DEFER_TAIL) so its
PE ops (bc, proj) don't stall on V/S queue latency.

PSUM budget (8 banks): 3 x [128,1024] score tiles (6 banks, also reused
as stage-A qkv/v-transpose scratch) + 1 shared attnv accumulator bank +
1 bank for the bc/pp normalize/projection tail.

Matmul dtypes: bf16 throughout the matmul path (FWL fast-weight-load
only engages for non-fp32 weights).
"""

import os

import numpy as np

import concourse.bass as bass
import concourse.mybir as mybir
import concourse.tile as tile
from concourse import bacc

F32 = mybir.dt.float32
F32R = mybir.dt.float32r
BF16 = mybir.dt.bfloat16
I16 = mybir.dt.int16
EXPF = mybir.ActivationFunctionType.Exp
COPYF = mybir.ActivationFunctionType.Copy

P = 128
B, N_FULL, C, H, D = 4, 2048, 256, 16, 16
CC = C // P  # 2 channel tiles
NCORES = 8

MM_DT = "bf16"    # qkv projection matmuls + x/weights (FWL weight path)
AV_DT = "bf16"    # attnv matmuls (col-group tile_position needs bf16 weights)
PROJ_DT = "bf16"  # sum-broadcast + output projection matmuls (FWL weight path)

_DT = {"f32r": F32R, "f32": F32, "bf16": BF16}

# Schraudolph int16/bf16 exp constants: bits = round(SC_A*s + SC_B);
# bitcast(bits) ~= exp(s), rel err rms ~1.8%, max ~4.2% (HW-validated).
_LOG2E = 1.4426950408889634
SC_A = 128.0 * _LOG2E
SC_B = 128.0 * 127.0 - 7.4

USE_DMA_T = True  # x transpose via DMA xbar (host passes x pre-transposed)

# EXP_MODE: "real" (normal) | "none" (timing ablation: attnv consumes a
# constant SBUF tile; output is numerically WRONG, only for attribution)
EXP_MODE = os.environ.get("EXP_MODE", "real")
TAIL_MODE = os.environ.get("TAIL_MODE", "real")  # "none": timing ablation
# Score-tile PSUM dtype: bf16 halves the tile to 1 bank (ring of 6 = 3kt
# of lookahead) and makes every exp operand 2-byte, enabling the DVE
# 2x_1p fast mode even from PSUM.  Scores are single-shot matmuls (no
# cross-matmul accumulation), so bf16 PSUM only costs output rounding.
SC_DT = os.environ.get("SC_DT", "f32")  # bf16 PSUM needs TRN3
# Score-tile granularity: "half" = per-head [128,512] tiles (1 PSUM bank
# each, ring of 6, finer head-of-line); "full" = [128,1024] 2-head tiles.
SC_GRAN = os.environ.get("SC_GRAN", "full")
# Exp-tile engine assignment: "greedy" (min completion time) or "alt"
# (strict S/V alternation per kt: avoids ring head-of-line convoys).
EXP_ASSIGN = os.environ.get("EXP_ASSIGN", "alt")

# ---- pipeline / scheduling knobs ----
AV_LAG = 2        # attnv matmuls trail the scores matmuls by this many kt
DEFER_TAIL = True       # emit each group's normalize/proj tail inside the
                        # next group's kt loop (hides PE tail stalls)
FLUSH_KT = 4      # kt index in the next group at which the tail is emitted

# Modeled per-op engine-busy costs (ns) used by the greedy scheduler.
# S=ScalarE(Act) V=VectorE(DVE) G=GPSIMD(Pool).  GPSIMD cannot access
# PSUM, so every op with a PSUM operand is restricted to S/V; GPSIMD
# gets the SBUF-only normalize multiply.
EXP_COST = {"S": 1040, "V": 1190}
BIAS_COST = {"S": 612}  # keep V pure-exp: V is the critical engine
VAUG_COST = {"S": 292}
ATCP_COST = {"S": 612}
OB_COST = {"S": 398}
REC_COST = 660
MUL_COST = {"V": 660, "G": 1111}

_NC_CACHE: dict = {}
LAST_RESULT = None  # BassKernelResults of the most recent run (for test.py)
TIMING_REPS = 1  # >1 repeats the compute on-device (timing); output unchanged


class _Sched:
    """Greedy min-completion-time assignment of elementwise ops to the
    three non-PE compute engines, using the modeled costs above."""

    def __init__(self):
        self.busy = {"S": 0.0, "V": 0.0, "G": 0.0}

    def pick(self, costs):
        e = min(costs, key=lambda k: self.busy[k] + costs[k])
        self.busy[e] += costs[e]
        return e

    def charge(self, e, cost):
        self.busy[e] += cost


def build(n_tokens=N_FULL, reps=1):
    N = n_tokens
    KT = N // P   # key tiles
    QC = 512      # q-chunk (psum bank = 512 fp32)
    NQ = N // QC
    TT = N // P   # token tiles

    MD = _DT[MM_DT]
    AD = _DT[AV_DT]
    PD = _DT[PROJ_DT]

    nc = bacc.Bacc()
    if USE_DMA_T:
        # host passes x already transposed: [C, N]
        x_d = nc.dram_tensor("x", [C, N], MD, kind="ExternalInput")
    else:
        x_d = nc.dram_tensor("x", [N, C], F32, kind="ExternalInput")
    wq_d = nc.dram_tensor("wq", [2, C, P], MD, kind="ExternalInput")
    wk_d = nc.dram_tensor("wk", [2, C, P], MD, kind="ExternalInput")
    wv_d = nc.dram_tensor("wv", [C, P], MD, kind="ExternalInput")
    bq_d = nc.dram_tensor("bq", [2, P], F32, kind="ExternalInput")
    bk_d = nc.dram_tensor("bk", [2, P], F32, kind="ExternalInput")
    bv_d = nc.dram_tensor("bv", [P], F32, kind="ExternalInput")
    wp_d = nc.dram_tensor("wp", [2, P, C], PD, kind="ExternalInput")
    sel_d = nc.dram_tensor("sel", [P, P], PD, kind="ExternalInput")
    idn_d = nc.dram_tensor("idn", [P, P], F32, kind="ExternalInput")
    out_d = nc.dram_tensor("out", [N, C], F32, kind="ExternalOutput")

    with tile.TileContext(nc) as tc:
        with (
            tc.tile_pool(name="const", bufs=1) as const,
            tc.tile_pool(name="work", bufs=12) as work,
            tc.tile_pool(name="ptp", bufs=(20 if SC_GRAN == "half" else 10)) as ptp,
            # Ring of score tiles plus stage-A psum scratch (qkv,
            # v-transpose).  "half" granularity: 1-bank tiles, ring of 6.
            tc.tile_pool(name="ps_s", bufs=(6 if SC_GRAN == "half" else 3),
                         space="PSUM") as ps_s,
            # All 4 attnv accumulation chains share ONE bank.
            tc.tile_pool(name="ps_at", bufs=1, space="PSUM") as ps_at,
            # bc/pp normalize+projection scratch: keeps the tail off the
            # hot sc ring.
            tc.tile_pool(name="ps_sm", bufs=1, space="PSUM") as ps_sm,
        ):
            # ---------------- loads ----------------
            def staged_load(name, shape, dt, src_ap):
                sb = const.tile(shape, dt, name=f"{name}_sb")
                nc.sync.dma_start(sb[:], src_ap)
                return sb

            # Small constants first: the first qkv matmuls wait on the
            # weights — queuing them behind the 2 MB x transfer costs
            # ~10 us of PE idle at startup.
            idn_sb = staged_load("idn", [P, P], F32, idn_d[:])
            wq_sb = staged_load(
                "wq", [P, 2, CC, P], MD,
                wq_d[:].rearrange("g (cc p) f -> p g cc f", p=P),
            )
            wk_sb = staged_load(
                "wk", [P, 2, CC, P], MD,
                wk_d[:].rearrange("g (cc p) f -> p g cc f", p=P),
            )
            wv_sb = staged_load(
                "wv", [P, CC, P], MD, wv_d[:].rearrange("(cc p) f -> p cc f", p=P)
            )
            if USE_DMA_T:
                # x arrives host-transposed [C, N]: DMA straight into the
                # [channels-on-partitions] layout, no on-chip transposes.
                # Chunk 0 queued before the remaining small constants so
                # the first projections unblock as early as possible.
                xt_full = const.tile([P, CC, N], MD, name="xt_full")
                x_r = x_d[:].rearrange("(cc p) t -> p cc t", p=P)
                for cc in range(CC):
                    nc.sync.dma_start(
                        xt_full[:, cc, 0:QC], x_r[:, cc, 0:QC]
                    )
            bq_sb = staged_load("bq", [P, 2], F32, bq_d[:].rearrange("g p -> p g"))
            bk_sb = staged_load("bk", [P, 2], F32, bk_d[:].rearrange("g p -> p g"))
            bv_sb = staged_load(
                "bv", [P, 1], F32, bv_d[:].rearrange("(p o) -> p o", o=1)
            )
            wp_sb = staged_load("wp", [P, 2, C], PD, wp_d[:].rearrange("g p c -> p g c"))
            sel_sb = staged_load("sel", [P, P], PD, sel_d[:])

            if USE_DMA_T:
                for cc in range(CC):
                    for qq in range(1, NQ):
                        nc.sync.dma_start(
                            xt_full[:, cc, qq * QC : (qq + 1) * QC],
                            x_r[:, cc, qq * QC : (qq + 1) * QC],
                        )
                x_sb = None
            else:
                x_sb = const.tile([P, TT, C], F32)
                x_r = x_d[:].rearrange("(t p) c -> p t c", p=P)
                for tt in range(TT):
                    nc.sync.dma_start(x_sb[:, tt, :], x_r[:, tt, :])
                xt_full = None

            from contextlib import nullcontext

            loop_ctx = tc.For_i(0, reps, 1) if reps > 1 else nullcontext()
            with loop_ctx:
                _build_body(
                    nc, tc, const, work, ptp, ps_s, ps_at, ps_sm,
                    N, KT, QC, NQ, TT, MD, AD, PD,
                    x_sb, xt_full, wq_sb, wk_sb, wv_sb, wp_sb, sel_sb, idn_sb,
                    bq_sb, bk_sb, bv_sb, out_d,
                )
    nc.finalize()
    return nc


def _build_body(
    nc, tc, const, work, ptp, ps_s, ps_at, ps_sm,
    N, KT, QC, NQ, TT, MD, AD, PD,
    x_sb, xt_full, wq_sb, wk_sb, wv_sb, wp_sb, sel_sb, idn_sb,
    bq_sb, bk_sb, bv_sb, out_d,
):
    ps_m = ps_s
    sched = _Sched()
    ones_sb = const.tile([P, 1], F32)
    nc.vector.memset(ones_sb[:], 1.0)
    # bf16 zero row + ones row for the full-width at-bank zero-fill matmul
    # (bf16 keeps the 512-col clear at 1 cycle/row on the PE).
    zrow_sb = const.tile([1, P], BF16)
    nc.vector.memset(zrow_sb[:], 0.0)
    onesrow_sb = const.tile([1, QC], BF16)
    nc.vector.memset(onesrow_sb[:], 1.0)
    KC = QC // P  # key tiles per chunk
    # q/k stored bf16: scores matmuls then use the FWL bf16 weight path.
    qt_t = [const.tile([P, 2, QC], AD, name=f"qt{c}") for c in range(NQ)]
    kt_t = [const.tile([P, 2, QC], AD, name=f"kt{c}") for c in range(NQ)]
    vt_t = [const.tile([P, QC], F32, name=f"vt{c}") for c in range(NQ)]
    vaug_t = [
        const.tile([P, KC, 8, 17], AD, name=f"vaug{c}") for c in range(NQ)
    ]
    if xt_full is None:
        xt_t = [const.tile([P, CC, QC], MD, name=f"xt{c}") for c in range(NQ)]
    else:
        xt_t = None

    # ot_raw ping-pong buffers (written by a full [128,512] copy of the
    # at bank; rows the chains never touched are exact zeros from the
    # full-width clear matmul).
    ot_raw_pp = [const.tile([P, QC], PD, name=f"otraw{i}") for i in range(2)]

    def xt_ap(c):
        if xt_full is not None:
            return xt_full[:, :, c * QC : (c + 1) * QC]
        return xt_t[c][:]

    def emit_bias_copy(dslice, ps, b_ap):
        e = sched.pick(BIAS_COST)
        if e == "S":
            # Identity (not Copy) accepts an AP bias; both live in the same
            # activation-table set as Exp, so no table-swap cost.
            nc.scalar.activation(
                dslice, ps[:], mybir.ActivationFunctionType.Identity, bias=b_ap
            )
        elif e == "V":
            nc.vector.tensor_scalar_add(dslice, ps[:], b_ap)
        else:
            nc.gpsimd.tensor_scalar_add(dslice, ps[:], b_ap)

    def emit_copy(dst, src, costs):
        e = sched.pick(costs)
        if e == "S":
            nc.scalar.activation(dst, src, COPYF)
        elif e == "V":
            nc.vector.tensor_copy(dst, src)
        else:
            nc.gpsimd.tensor_copy(dst, src)

    for c in range(NQ):
        if xt_full is None:
            # xT for this chunk via PE transpose
            for ti in range(QC // P):
                tt = c * (QC // P) + ti
                for cc in range(CC):
                    tp = ps_m.tile([P, P], F32, tag="scores", name="tp")
                    nc.tensor.transpose(
                        tp[:], x_sb[:, tt, cc * P : (cc + 1) * P], idn_sb[:]
                    )
                    nc.vector.tensor_copy(
                        xt_t[c][:, cc, ti * P : (ti + 1) * P], tp[:]
                    )
        xc = xt_ap(c)
        # k, v (needed for all q-chunks) then q projections
        projs = [
            (wk_sb[:, 0], bk_sb[:, 0:1], kt_t[c][:, 0]),
            (wk_sb[:, 1], bk_sb[:, 1:2], kt_t[c][:, 1]),
            (wv_sb[:], bv_sb[:, 0:1], vt_t[c][:]),
            (wq_sb[:, 0], bq_sb[:, 0:1], qt_t[c][:, 0]),
            (wq_sb[:, 1], bq_sb[:, 1:2], qt_t[c][:, 1]),
        ]
        for w_ap, b_ap, dslice in projs:
            ps = ps_m.tile([P, QC], F32, tag="scores", name="ps")
            for cc in range(CC):
                nc.tensor.matmul(
                    ps[:],
                    w_ap[:, cc, :],
                    xc[:, cc, :],
                    start=(cc == 0),
                    stop=(cc == CC - 1),
                )
            # single fused PSUM->SBUF copy + per-partition bias add
            emit_bias_copy(dslice, ps, b_ap)
        # v_aug for this chunk (v natural layout + ones column)
        nc.vector.tensor_copy(
            vaug_t[c][:, :, :, 16],
            ones_sb[:, 0:1, None].to_broadcast((P, KC, 8)),
        )
        for ki in range(KC):
            tp = ps_m.tile([P, P], F32, tag="scores", name="tp")
            nc.tensor.transpose(
                tp[:], vt_t[c][:, ki * P : (ki + 1) * P], idn_sb[:]
            )
            emit_copy(
                vaug_t[c][:, ki, :, 0:16],
                tp[:].rearrange("p (h d) -> p h d", d=16),
                VAUG_COST,
            )

    # ---------------- attention ----------------
    # One flat software-pipelined stream over (nn, g2, kt).  The attnv
    # matmuls trail the scores matmuls by AV_LAG kt-steps and the pend
    # queue is carried ACROSS group boundaries, so the pipeline never
    # drains: the next group's scores are emitted before the previous
    # group's last attnvs.  Each group's at-bank zero-fill matmul is
    # emitted lazily, just before the group's first attnv.  The
    # normalize/projection tails and the q-projections for chunks 1..3
    # are spread into the stream (one item per even kt slot).
    pending_tail = []  # list of small closures, flushed one per kt slot

    def flush_tail():
        while pending_tail:
            pending_tail.pop(0)()

    def flush_one():
        if pending_tail:
            pending_tail.pop(0)()

    def emit_sc(nn, g2, kt):
        if SC_GRAN == "half":
            scs = []
            for lj in range(4):
                rg = 32 * lj
                sc = ps_s.tile([P, QC], SCD, tag="scores", name="sc")
                nc.tensor.matmul(
                    sc[:],
                    kt_t[kt // KC][
                        rg : rg + D, g2,
                        (kt % KC) * P : (kt % KC + 1) * P,
                    ],
                    qt_t[nn][rg : rg + D, g2, :],
                    start=True,
                    stop=True,
                    tile_position=(rg, 0),
                )
                scs.append(sc)
            return scs
        scs = []
        for pr in range(2):
            sc = ps_s.tile([P, 2 * QC], SCD, tag="scores", name="sc")
            for j2 in range(2):
                lj = 2 * pr + j2
                rg = 32 * lj
                nc.tensor.matmul(
                    sc[:, j2 * QC : (j2 + 1) * QC],
                    kt_t[kt // KC][
                        rg : rg + D, g2,
                        (kt % KC) * P : (kt % KC + 1) * P,
                    ],
                    qt_t[nn][rg : rg + D, g2, :],
                    start=True,
                    stop=True,
                    tile_position=(rg, 0),
                )
            scs.append(sc)
        return scs

    SCD = BF16 if SC_DT == "bf16" else F32

    dummy_pt = None
    if EXP_MODE == "none":
        dummy_pt = const.tile([P, 2 * QC], AD, name="dummy_pt")
        nc.vector.memset(dummy_pt[:], 0.001)
        if SC_GRAN == "half":
            dummy_pt = dummy_pt[:, 0:QC]

    def emit_exp(sc, pr=0):
        if EXP_MODE == "none":
            return dummy_pt[:] if not isinstance(dummy_pt, type(sc)) else dummy_pt
        wid = QC if SC_GRAN == "half" else 2 * QC
        if EXP_ASSIGN == "alt":
            e = "S" if pr % 2 == 0 else "V"
            sched.charge(e, EXP_COST[e])
        else:
            e = sched.pick(EXP_COST)
        if e == "S":
            pt = ptp.tile([P, wid], AD, tag="pt", name="pt")
            nc.scalar.activation(pt[:], sc[:], EXPF)
            return pt[:]
        pt = ptp.tile([P, wid], I16, tag="pt", name="pt")
        eng = nc.vector if e == "V" else nc.gpsimd
        eng.tensor_scalar(
            pt[:], sc[:], SC_A, SC_B,
            mybir.AluOpType.mult, mybir.AluOpType.add,
        )
        return pt[:].bitcast(BF16)

    def emit_av(at, g2, kt, pts):
        for lj in range(4):
            if SC_GRAN == "half":
                mv = pts[lj]
            else:
                pr, j2 = lj // 2, lj % 2
                mv = pts[pr][:, j2 * QC : (j2 + 1) * QC]
            nc.tensor.matmul(
                at[32 * lj : 32 * lj + 17, :],
                vaug_t[kt // KC][:, kt % KC, 4 * g2 + lj, :],
                mv,
                start=False,
                stop=(kt == KT - 1),
                tile_position=(0, 32 * lj),
            )

    def mk_qproj(c):
        def qproj():
            xc = xt_ap(c)
            for gq in range(2):
                ps = ps_m.tile([P, QC], F32, tag="scores", name="ps")
                for cc in range(CC):
                    nc.tensor.matmul(
                        ps[:],
                        wq_sb[:, gq, cc, :],
                        xc[:, cc, :],
                        start=(cc == 0),
                        stop=(cc == CC - 1),
                    )
                emit_bias_copy(qt_t[c][:, gq], ps, bq_sb[:, gq : gq + 1])
        return qproj

    for c in range(1, NQ):
        pending_tail.append(mk_qproj(c))

    groups = [(nn, g2) for nn in range(NQ) for g2 in range(2)]
    ot_ns = {}
    pend = []  # (at, g2, kt, pts) awaiting attnv
    group_state = {}  # gi -> (at tile, cleared?)

    def get_at(gi):
        if gi not in group_state:
            at = ps_at.tile([P, QC], F32, tag="at", name="at")
            nc.tensor.matmul(
                at[:], zrow_sb[:], onesrow_sb[:], start=True, stop=True,
            )
            group_state[gi] = at
        return group_state[gi]

    def drain_one():
        at0, g20, k0, p0 = pend.pop(0)
        emit_av(at0, g20, k0, p0)

    for gi, (nn, g2) in enumerate(groups):
        if g2 == 0:
            ot_ns[nn] = work.tile([P, 2, QC], PD, tag="otn", name="otn")
        ot_n = ot_ns[nn]
        for kt in range(KT):
            scs = emit_sc(nn, g2, kt)
            pts = [emit_exp(sc_i, i) for i, sc_i in enumerate(scs)]
            # group's at bank is allocated/cleared lazily, right before
            # its first attnv (so the clear overlaps the previous
            # group's stream, not the pipeline refill).
            pend.append((None, g2, kt, pts))
            if len(pend) > AV_LAG:
                at0, g20, k0, p0 = pend.pop(0)
                emit_av(get_at(gi if k0 <= kt else gi - 1), g20, k0, p0)
            # flush AFTER the drain: the at-copy tail item of group gi-1
            # must be emitted after that group's last attnv (kt=1 slot).
            if kt >= 1 and kt % 2 == 1:
                flush_one()

        # resolve which group each pending av belongs to: entries with
        # kt close to KT-1 are this group's; rewrite with the at handle
        # once known.  (All avs for group gi use get_at(gi).)
        # Tail bookkeeping: emitted when this group's last av drains --
        # see below; we tag the tail closures now.
        ot_raw = ot_raw_pp[gi % 2]

        def mk_atcopy(gi=gi, ot_raw=ot_raw):
            def atcopy():
                emit_copy(ot_raw[:], group_state[gi][:], ATCP_COST)
            return atcopy

        def mk_norm(nn=nn, g2=g2, ot_raw=ot_raw, ot_n=ot_n):
            def norm():
                bc = ps_sm.tile([P, QC], F32, tag="small", name="bc")
                nc.tensor.matmul(
                    bc[:], sel_sb[:], ot_raw[:], start=True, stop=True
                )
                rec = work.tile([P, QC], F32, tag="rec")
                nc.vector.reciprocal_approx_fast(rec[:], bc[:])
                sched.charge("V", REC_COST)
                e = sched.pick(MUL_COST)
                eng = nc.vector if e == "V" else nc.gpsimd
                eng.tensor_mul(ot_n[:, g2, :], ot_raw[:], rec[:])
            return norm

        def mk_proj(nn=nn, ot_n=ot_n, ss=0):
            def proj():
                pp = ps_sm.tile([P, C], F32, tag="small", name="pp")
                for gg in range(2):
                    nc.tensor.matmul(
                        pp[:],
                        ot_n[:, gg, ss * P : (ss + 1) * P],
                        wp_sb[:, gg, :],
                        start=(gg == 0),
                        stop=(gg == 1),
                    )
                ob = work.tile([P, C], F32, tag="ob")
                emit_copy(ob[:], pp[:], OB_COST)
                tt_idx = nn * (QC // P) + ss
                nc.sync.dma_start(
                    out_d[:].rearrange("(t p) c -> p t c", p=P)[
                        :, tt_idx, :
                    ],
                    ob[:],
                )
            return proj

        if TAIL_MODE != "none":
            pending_tail.append(mk_atcopy())
            pending_tail.append(mk_norm())
            if g2 == 1:
                for ss in range(QC // P):
                    pending_tail.append(mk_proj(ss=ss))
    # drain remaining avs and tails
    while pend:
        at0, g20, k0, p0 = pend.pop(0)
        emit_av(get_at(len(groups) - 1), g20, k0, p0)
    flush_tail()

def _get_nc(n_tokens=N_FULL, reps=1):
    key = (n_tokens, MM_DT, AV_DT, PROJ_DT, USE_DMA_T, reps,
           AV_LAG, DEFER_TAIL, FLUSH_KT, EXP_MODE, TAIL_MODE, SC_DT, EXP_ASSIGN,
           SC_GRAN,
           tuple(sorted(EXP_COST.items())))
    if key not in _NC_CACHE:
        _NC_CACHE[key] = build(n_tokens, reps=reps)
    return _NC_CACHE[key]


def make_core_inputs(core, x, w_qkv, b_qkv, w_proj, n_tokens=N_FULL):
    """Host-side sharding: slice/spread weights for one core."""
    b, g = core // 2, core % 2
    wq_s = np.zeros((2, C, P), np.float32)
    wk_s = np.zeros((2, C, P), np.float32)
    bq_s = np.zeros((2, P), np.float32)
    bk_s = np.zeros((2, P), np.float32)
    wv_s = np.zeros((C, P), np.float32)
    bv_s = np.zeros((P,), np.float32)
    wp_s = np.zeros((2, P, C), np.float32)
    for g2 in range(2):
        for j in range(4):
            h = 8 * g + 4 * g2 + j
            sp = slice(32 * j, 32 * j + D)
            wq_s[g2, :, sp] = w_qkv[:, 0 * C + h * D : 0 * C + (h + 1) * D]
            wk_s[g2, :, sp] = w_qkv[:, 1 * C + h * D : 1 * C + (h + 1) * D]
            bq_s[g2, sp] = b_qkv[0 * C + h * D : 0 * C + (h + 1) * D]
            bk_s[g2, sp] = b_qkv[1 * C + h * D : 1 * C + (h + 1) * D]
            wp_s[g2, sp, :] = w_proj[h * D : (h + 1) * D, :]
    for lh in range(8):
        h = 8 * g + lh
        wv_s[:, 16 * lh : 16 * lh + 16] = w_qkv[:, 2 * C + h * D : 2 * C + (h + 1) * D]
        bv_s[16 * lh : 16 * lh + 16] = b_qkv[2 * C + h * D : 2 * C + (h + 1) * D]
    sel = np.zeros((P, P), np.float32)
    for j in range(4):
        sel[32 * j + 16, 32 * j : 32 * j + 32] = 1.0
    idn = np.eye(P, dtype=np.float32)

    def cast(a, stage_dt):
        if stage_dt == "bf16":
            import ml_dtypes
            return a.astype(ml_dtypes.bfloat16)
        return a.astype(np.float32)

    if USE_DMA_T:
        x_core = cast(np.ascontiguousarray(x[b, :n_tokens].T), MM_DT)
    else:
        x_core = np.ascontiguousarray(x[b, :n_tokens], dtype=np.float32)
    return {
        "x": x_core,
        "wq": cast(wq_s, MM_DT), "wk": cast(wk_s, MM_DT), "wv": cast(wv_s, MM_DT),
        "bq": bq_s, "bk": bk_s, "bv": bv_s,
        "wp": cast(wp_s, PROJ_DT), "sel": cast(sel, PROJ_DT), "idn": idn,
    }


def kernel(x, w_qkv, b_qkv, w_proj, b_proj):
    global LAST_RESULT
    from concourse.bass_utils import run_bass_kernel_spmd

    x = np.asarray(x, dtype=np.float32)
    w_qkv = np.asarray(w_qkv, dtype=np.float32)
    b_qkv = np.asarray(b_qkv, dtype=np.float32)
    w_proj = np.asarray(w_proj, dtype=np.float32)
    b_proj = np.asarray(b_proj, dtype=np.float32)

    nc = _get_nc(reps=TIMING_REPS)
    in_maps = [
        make_core_inputs(core, x, w_qkv, b_qkv, w_proj) for core in range(NCORES)
    ]
    res = run_bass_kernel_spmd(nc, in_maps, list(range(NCORES)))
    LAST_RESULT = res
    out = np.zeros((B, N_FULL, C), np.float32)
    for core in range(NCORES):
        out[core // 2] += res.results[core]["out"]
    out += b_proj[None, None, :]
    return out
